# revision 25
# baseline (speedup 1.0000x reference)
"""Trainium2 Bass kernel for nn_Model_29592324670139 (dense transformer).

Sharding: 8 cores = 4 pairs. Pair b handles batch item b; within a pair the
672-token sequence (21 vars x 32 windows, window-major order) is split by
window parity (rank0 = even windows, rank1 = odd windows), 336 tokens each.
Host pre-normalizes x (instance norm) and finishes the head+MLP from
per-core partial pooled sums. Per layer each core projects Q/K/V for its
tokens; K/V are AllGathered within the pair (one collective each);
attention/FFN/LN run on local tokens.

Activations are feature-major ([d, token], d on partitions), bf16 matmuls
with fp32 PSUM; softmax/LN statistics in fp32.

Self-contained: hardcodes all shapes; only needs numpy/ml_dtypes/concourse.
"""

import numpy as np
import ml_dtypes

import concourse.bass as bass
import concourse.tile as tile
from concourse import bacc, mybir
from concourse.bass import ts, ds
from concourse.bass_utils import run_bass_kernel_spmd

F32 = mybir.dt.float32
BF16 = mybir.dt.bfloat16
AX = mybir.AluOpType
AF = mybir.ActivationFunctionType
XL = mybir.AxisListType

B, L, C = 4, 3072, 21
P, OUT, D, H, NL, DFF = 96, 96, 1024, 16, 2, 4096
SL = 336         # local tokens per core
S = 672          # full sequence
HD = 64          # head dim
NKC = D // 128   # 8 k-chunks of d_model
NFC = DFF // 128  # 32 chunks of d_ff

REPLICA_GROUPS = [[0, 1], [2, 3], [4, 5], [6, 7]]

# rank-invariant query-suffix starts per gathered key chunk (block-causal)
SUF0 = [0, 105, 210, 0, 105, 210]
SUF = [SL - t for t in SUF0]
# score chunks paired into single [112, 462] PSUM banks; esb packed to match
PAIRS = [(0, 2), (3, 5), (1, 4)]
EOFF = {0: 0, 2: 336, 3: 462, 5: 798, 1: 924, 4: 1155}
EW = 1386

KREG = 128 * NKC * SL               # K payload elems per rank
VSZ = 112 * 3 * (H * (HD + 1))      # V payload elems per rank (ones incl.)

_BUILT = None
LAST_RESULT = None
DEBUG = False


# ----------------------------------------------------------------------------
# device program
# ----------------------------------------------------------------------------

def _build():
    nc = bacc.Bacc("TRN2", target_bir_lowering=False, debug=False,
                   enable_asserts=False, num_devices=8)
    t = {}

    def din(name, shape, dt):
        t[name] = nc.dram_tensor(name, list(shape), dt, kind="ExternalInput").ap()

    din("xloc", (P, SL), BF16)
    din("maskP", (112, EW), BF16)
    din("embW", (P, D), BF16)
    din("biases", (128, NKC + NL * (8 * NKC + NFC)), F32)
    din("bvpack", (1, NL * D), BF16)
    for l in range(NL):
        for w in ("Wk", "Wv", "Wq", "Wo"):
            din(f"{w}{l}", (128, NKC, D), BF16)       # [p, kc, n]
        din(f"W1{l}", (8, 128, NKC, 512), BF16)       # eighths [j, p, kc, n]
        din(f"W2{l}", (NKC, 128, NFC, 128), BF16)     # per-oc [oc, p, kc, c]

    out_dram = nc.dram_tensor("out", [129, 8], F32, kind="ExternalOutput").ap()
    dbg = {}
    if DEBUG:
        for name, shape in [("dbg_h0", (128, NKC, SL)), ("dbg_k", (128, NKC, S)),
                            ("dbg_q", (128, NKC, SL)),
                            ("dbg_v", (112, 6, H, HD + 1)),
                            ("dbg_esb0", (112, EW)), ("dbg_esb5", (112, EW)),
                            ("dbg_exp0", (112, EW)), ("dbg_mask", (112, EW)),
                            ("dbg_att", (128, NKC, SL)),
                            ("dbg_d1", (128, NKC, SL)), ("dbg_h1", (128, NKC, SL)),
                            ("dbg_g", (128, NFC, SL)), ("dbg_h2", (128, NKC, SL)),
                            ("dbg_h3", (128, NKC, SL))]:
            dbg[name] = nc.dram_tensor(name, list(shape), BF16,
                                       kind="ExternalOutput").ap()
        for name, shape in [("dbg_psa", (HD + 1, SL)), ("dbg_rec", (1, SL)),
                            ("dbg_rb", (64, SL))]:
            dbg[name] = nc.dram_tensor(name, list(shape), F32,
                                       kind="ExternalOutput").ap()

    with tile.TileContext(nc) as tc:
        _emit(tc, t, out_dram, dbg)

    nc.compile()
    return nc, set(t.keys())


def _emit(tc, t, out_dram, dbg=None):
    from contextlib import ExitStack

    def dump(name, tile_ap):
        if dbg and name in dbg:
            tc.nc.sync.dma_start(out=dbg[name][:], in_=tile_ap)
    nc = tc.nc
    ctx = ExitStack()

    constp = ctx.enter_context(tc.tile_pool(name="constp", bufs=1))
    wqkv = ctx.enter_context(tc.tile_pool(name="wqkv", bufs=1))
    w1p = ctx.enter_context(tc.tile_pool(name="w1p", bufs=2))
    w2p = ctx.enter_context(tc.tile_pool(name="w2p", bufs=2))
    esbp = ctx.enter_context(tc.tile_pool(name="esbp", bufs=7))
    sqp = ctx.enter_context(tc.tile_pool(name="sqp", bufs=2))
    tmpp = ctx.enter_context(tc.tile_pool(name="tmpp", bufs=2))
    lnp = ctx.enter_context(tc.tile_pool(name="lnp", bufs=1))
    recp = ctx.enter_context(tc.tile_pool(name="recp", bufs=2))
    denp = ctx.enter_context(tc.tile_pool(name="denp", bufs=1))
    dramp = ctx.enter_context(tc.tile_pool(name="dramp", bufs=1, space="DRAM"))
    psMM = ctx.enter_context(tc.tile_pool(name="psMM", bufs=2, space="PSUM"))
    psS = ctx.enter_context(tc.tile_pool(name="psS", bufs=3, space="PSUM"))
    psAV = ctx.enter_context(tc.tile_pool(name="psAV", bufs=2, space="PSUM"))
    psST = ctx.enter_context(tc.tile_pool(name="psST", bufs=1, space="PSUM"))

    def single(shape, dt, name, **kw):
        tl, free = tc.tile(shape, dt, name=name, **kw)
        ctx.callback(free)
        return tl

    # ---------------- front loads ----------------
    xloc_sb = constp.tile([P, SL], BF16, name="xloc_sb", tag="xloc_sb")
    nc.sync.dma_start(out=xloc_sb[:], in_=t["xloc"][:])
    embW_sb = constp.tile([P, D], BF16, name="embW_sb", tag="embW_sb")
    nc.sync.dma_start(out=embW_sb[:], in_=t["embW"][:])
    wk = wqkv.tile([128, NKC, D], BF16, name="wk", tag="wk")
    nc.scalar.dma_start(out=wk[:, 0:4, :], in_=t["Wk0"][:, 0:4, :])
    nc.sync.dma_start(out=wk[:, 4:8, :], in_=t["Wk0"][:, 4:8, :])
    wq = wqkv.tile([128, NKC, D], BF16, name="wq", tag="wq")
    nc.scalar.dma_start(out=wq[:], in_=t["Wq0"][:])
    NB = NKC + NL * (8 * NKC + NFC)
    sb_bias = constp.tile([128, NB], F32, name="sb_bias", tag="sb_bias")
    nc.sync.dma_start(out=sb_bias[:], in_=t["biases"][:])
    sb_mask = constp.tile([112, EW], BF16, name="sb_mask", tag="sb_mask")
    nc.sync.dma_start(out=sb_mask[:], in_=t["maskP"][:])
    wv = wqkv.tile([128, NKC, D], BF16, name="wv", tag="wv")
    nc.sync.dma_start(out=wv[:], in_=t["Wv0"][:])
    wo = wqkv.tile([128, NKC, D], BF16, name="wo", tag="wk")
    nc.sync.dma_start(out=wo[:], in_=t["Wo0"][:])

    _bc = [0]

    def bias_col(n=NKC):
        c0 = _bc[0]
        _bc[0] += n
        return sb_bias[:, c0:c0 + n]

    sb_embB = bias_col()
    bias_sb = {}
    for l in range(NL):
        for v in ("bq", "bk", "bo", "b2", "ln1s", "ln1b", "ln2s", "ln2b"):
            bias_sb[f"{v}{l}"] = bias_col()
        bias_sb[f"b1{l}"] = bias_col(NFC)

    ones_bf = constp.tile([128, 1], BF16, name="ones_bf", tag="ones_bf")
    nc.vector.memset(ones_bf[:], 1.0)
    ones64 = constp.tile([1, 64], BF16, name="ones64", tag="ones64")
    nc.vector.memset(ones64[:], 1.0)
    ones128r = constp.tile([1, 128], BF16, name="ones128r", tag="ones128r")
    nc.vector.memset(ones128r[:], 1.0)
    eps5_sb = constp.tile([1, 1], F32, name="eps5_sb", tag="eps5_sb")
    nc.vector.memset(eps5_sb[:], 1e-5)
    zero_sb = constp.tile([128, 1], F32, name="zero_sb", tag="zero_sb")
    nc.vector.memset(zero_sb[:], 0.0)

    # persistent activations
    h_bf = single([128, NKC, SL], BF16, "h_bf")
    d_bf = single([128, NKC, SL], BF16, "d_bf")
    q_sb = single([128, NKC, SL], BF16, "q_sb")
    k_loc = single([128, NKC, SL], BF16, "k_loc")
    v_loc = single([112, 3, H, HD + 1], BF16, "v_loc")
    k_full = single([128, NKC, S], BF16, "k_full")
    v_full = single([112, 6, H, HD + 1], BF16, "v_full")
    att_sb = single([128, NKC, SL], BF16, "att_sb")
    g_sb = single([128, NFC, SL], BF16, "g_sb")
    hsum = constp.tile([128, NKC], F32, name="hsum", tag="hsum")

    nc.vector.memset(v_loc[:, :, :, HD:HD + 1], 1.0)

    # tiny warm-up AllGather: pays the first-collective init cost during the
    # prologue instead of on layer 0's critical path
    wuin = dramp.tile([64], BF16, name="wuin", tag="wuin")
    wuout = single([128], BF16, "wuout", space="DRAM", addr_space="Shared")
    nc.scalar.dma_start(out=wuin[:], in_=t["xloc"][0:1, 0:64])
    nc.gpsimd.collective_compute(
        "AllGather", AX.bypass, replica_groups=REPLICA_GROUPS,
        ins=[wuin[:]], outs=[wuout[:]])

    # ---------------- embedding ----------------
    for oc in range(NKC):
        pse = psMM.tile([128, SL], F32, name="pse", tag="mm")
        nc.tensor.matmul(pse[:], lhsT=embW_sb[:, ts(oc, 128)], rhs=xloc_sb[:],
                         start=True, stop=True)
        nc.scalar.activation(out=h_bf[:, oc, :], in_=pse[:], func=AF.Identity,
                             bias=sb_embB[:, oc:oc + 1])
    dump("dbg_h0", h_bf[:])

    # ---------------- LN helper (stats + apply) ----------------
    def ln_stats(src):
        pstat = psST.tile([33, SL], F32, name="pstat", tag="st")
        sqs = []
        for kc in range(NKC):
            sq = sqp.tile([128, SL], BF16, name="sq", tag="sq")
            nc.scalar.activation(out=sq[:], in_=src[:, kc, :], func=AF.Square)
            sqs.append(sq)
        for kc in range(NKC):
            nc.tensor.matmul(pstat[0:1, :], lhsT=ones_bf[:], rhs=src[:, kc, :],
                             start=(kc == 0), stop=(kc == NKC - 1))
        for kc in range(NKC):
            nc.tensor.matmul(pstat[32:33, :], lhsT=ones_bf[:], rhs=sqs[kc][:],
                             start=(kc == 0), stop=(kc == NKC - 1))
        mean1 = lnp.tile([1, SL], F32, name="mean1", tag="mean1")
        nc.scalar.activation(out=mean1[:], in_=pstat[0:1, :], func=AF.Copy,
                             scale=1.0 / D)
        msq = lnp.tile([1, SL], F32, name="msq", tag="msq")
        nc.vector.tensor_mul(msq[:], mean1[:], mean1[:])
        var1 = lnp.tile([1, SL], F32, name="var1", tag="var1")
        nc.vector.scalar_tensor_tensor(out=var1[:], in0=pstat[32:33, :],
                                       scalar=1.0 / D, in1=msq[:],
                                       op0=AX.mult, op1=AX.subtract)
        nc.scalar.activation(out=msq[:], in_=var1[:], func=AF.Sqrt,
                             bias=eps5_sb[:])
        r1 = lnp.tile([1, SL], F32, name="r1", tag="var1")
        nc.vector.reciprocal_approx_fast(out=r1[:], in_=msq[:])
        r_bf = lnp.tile([1, SL], BF16, name="r_bf", tag="r_bf")
        nc.vector.tensor_scalar_mul(r_bf[:], r1[:], 1.0)
        r_b = psS.tile([128, SL], F32, name="r_b", tag="s")
        nc.tensor.matmul(r_b[:], lhsT=ones128r[:], rhs=r_bf[:],
                         start=True, stop=True)
        return mean1, r1, r_b

    def ln(src, s_sb, b_sb):
        mean1, r1, r_b = ln_stats(src)
        mean_bf = lnp.tile([1, SL], BF16, name="mean_bf", tag="mean_bf")
        nc.scalar.activation(out=mean_bf[:], in_=mean1[:], func=AF.Copy)
        mean_b = psS.tile([128, SL], F32, name="mean_b", tag="s")
        nc.tensor.matmul(mean_b[:], lhsT=ones128r[:], rhs=mean_bf[:],
                         start=True, stop=True)
        for kc in range(NKC):
            t1 = tmpp.tile([128, SL], BF16, name="t1", tag="t1")
            nc.vector.tensor_sub(t1[:], src[:, kc, :], mean_b[:])
            u1 = tmpp.tile([128, SL], BF16, name="u1", tag="u1")
            nc.vector.scalar_tensor_tensor(
                out=u1[:], in0=t1[:], scalar=s_sb[:, kc:kc + 1], in1=r_b[:],
                op0=AX.mult, op1=AX.mult)
            nc.scalar.activation(out=h_bf[:, kc, :], in_=u1[:],
                                 func=AF.Identity, bias=b_sb[:, kc:kc + 1])

    # ---------------- transformer layers ----------------
    wtiles = {"wk": wk, "wv": wv, "wq": wq, "wo": wo}
    for l in range(NL):
        wk, wv, wq, wo = (wtiles[n] for n in ("wk", "wv", "wq", "wo"))

        # V bias: replicate [1, D] dram row to 112 partitions via DMA
        bv_b = constp.tile([112, D], BF16, name="bv_b", tag="bv_b")
        nc.sync.dma_start(out=bv_b[:], in_=bass.AP(
            tensor=t["bvpack"].tensor, offset=l * D, ap=[[0, 112], [1, D]]))

        # K projection -> k_loc, stage, AllGather
        bsb = bias_sb[f"bk{l}"]
        for oc in range(NKC):
            psp = psMM.tile([128, SL], F32, name="psp", tag="mm")
            for kc in range(NKC):
                nc.tensor.matmul(psp[:], lhsT=wk[:, kc, ts(oc, 128)],
                                 rhs=h_bf[:, kc, :],
                                 start=(kc == 0), stop=(kc == NKC - 1))
            nc.vector.tensor_scalar_add(k_loc[:, oc, :], psp[:],
                                        bsb[:, oc:oc + 1])
        kbnc_in = dramp.tile([KREG], BF16, name=f"kbnc_in{l}",
                             tag=f"kbnc_in{l}")
        kbnc_out = single([2 * KREG], BF16, f"kbnc_out{l}", space="DRAM",
                          addr_space="Shared")
        nc.sync.dma_start(
            out=kbnc_in[:].rearrange("(p kc tk) -> p kc tk", p=128, tk=SL),
            in_=k_loc[:])
        nc.gpsimd.collective_compute(
            "AllGather", AX.bypass, replica_groups=REPLICA_GROUPS,
            ins=[kbnc_in[:]], outs=[kbnc_out[:]])

        # V projection -> v_loc (token-major, ones col kept), stage, AllGather
        for tc3 in range(3):
            for nh in range(2):
                psv = psMM.tile([112, 512], F32, name="psv", tag="mm")
                for kc in range(NKC):
                    nc.tensor.matmul(psv[:], lhsT=h_bf[:, kc, ds(tc3 * 112, 112)],
                                     rhs=wv[:, kc, ts(nh, 512)],
                                     start=(kc == 0), stop=(kc == NKC - 1))
                nc.vector.tensor_add(
                    v_loc[:, tc3, ds(nh * 8, 8), 0:HD],
                    psv[:].rearrange("p (h e) -> p h e", e=HD),
                    bv_b[:, ts(nh, 512)].rearrange("p (h e) -> p h e", e=HD))
        vbnc_in = dramp.tile([VSZ], BF16, name=f"vbnc_in{l}",
                             tag=f"vbnc_in{l}")
        vbnc_out = single([2 * VSZ], BF16, f"vbnc_out{l}", space="DRAM",
                          addr_space="Shared")
        nc.sync.dma_start(
            out=vbnc_in[:].rearrange("(p c he) -> p c he", p=112,
                                     he=H * (HD + 1)),
            in_=v_loc[:].rearrange("p c h e -> p c (h e)"))
        nc.gpsimd.collective_compute(
            "AllGather", AX.bypass, replica_groups=REPLICA_GROUPS,
            ins=[vbnc_in[:]], outs=[vbnc_out[:]])

        # Q projection (overlaps the AllGathers)
        bsb = bias_sb[f"bq{l}"]
        for oc in range(NKC):
            psp = psMM.tile([128, SL], F32, name="psp", tag="mm")
            for kc in range(NKC):
                nc.tensor.matmul(psp[:], lhsT=wq[:, kc, ts(oc, 128)],
                                 rhs=h_bf[:, kc, :],
                                 start=(kc == 0), stop=(kc == NKC - 1))
            nc.vector.tensor_scalar_add(q_sb[:, oc, :], psp[:],
                                        bsb[:, oc:oc + 1])

        # gather-backs (gpsimd queue; wait on the AllGathers by data dep)
        for r in range(2):
            nc.sync.dma_start(
                out=k_full[:, :, ds(r * SL, SL)],
                in_=kbnc_out[ds(r * KREG, KREG)].rearrange(
                    "(p kc tk) -> p kc tk", p=128, tk=SL))
            nc.sync.dma_start(
                out=v_full[:, ds(3 * r, 3)].rearrange("p c h e -> p c (h e)"),
                in_=vbnc_out[ds(r * VSZ, VSZ)].rearrange(
                    "(p c he) -> p c he", p=112, he=H * (HD + 1)))

        # ---- attention: scores+exp windowed with mask/AV pipeline ----
        esbs = {}

        def scores_head(hh):
            hb2 = 64 * (hh % 2)
            hc = hh // 2
            esb = esbp.tile([112, EW], BF16, name="esb", tag="esb")
            esbs[hh] = esb
            for ca, cb in PAIRS:
                sa, sb2 = SUF[ca], SUF[cb]
                pss = psS.tile([112, 462], F32, name="pss", tag="s")
                nc.tensor.matmul(pss[:, 0:sa],
                                 lhsT=k_full[ds(hb2, 64), hc, ts(ca, 112)],
                                 rhs=q_sb[ds(hb2, 64), hc, ds(SUF0[ca], sa)],
                                 start=True, stop=False)
                nc.tensor.matmul(pss[:, ds(sa, sb2)],
                                 lhsT=k_full[ds(hb2, 64), hc, ts(cb, 112)],
                                 rhs=q_sb[ds(hb2, 64), hc, ds(SUF0[cb], sb2)],
                                 start=False, stop=True)
                nc.scalar.activation(out=esb[:, ds(EOFF[ca], sa + sb2)],
                                     in_=pss[:, 0:sa + sb2], func=AF.Exp,
                                     bias=zero_sb[0:112])
            if l == 0 and hh == 0:
                dump("dbg_exp0", esb[:])
                dump("dbg_mask", sb_mask[:])

        def finish_head(hh):
            hc = hh // 2
            esb = esbs[hh]
            for ca, cb in PAIRS:
                w2_ = SUF[ca] + SUF[cb]
                nc.vector.tensor_mul(esb[:, ds(EOFF[ca], w2_)],
                                     esb[:, ds(EOFF[ca], w2_)],
                                     sb_mask[:, ds(EOFF[ca], w2_)])
            if l == 0 and hh == 0:
                dump("dbg_esb0", esb[:])
            psa = psAV.tile([HD + 1, SL], F32, name="psa", tag="av")
            for i, cc in enumerate([0, 1, 2, 3, 4, 5]):
                tqs, suf = SUF0[cc], SUF[cc]
                nc.tensor.matmul(psa[:, ds(tqs, suf)],
                                 lhsT=v_full[:, cc, hh, :],
                                 rhs=esb[:, ds(EOFF[cc], suf)],
                                 start=(i == 0), stop=(i == 5))
            den = denp.tile([1, SL], F32, name="den", tag="den")
            if hh % 2 == 0:
                nc.scalar.activation(out=den[:], in_=psa[ds(HD, 1), :],
                                     func=AF.Copy)
            else:
                nc.vector.tensor_scalar_mul(den[:], psa[ds(HD, 1), :], 1.0)
            rec = recp.tile([1, SL], F32, name="rec", tag="rec")
            nc.vector.reciprocal_approx_fast(out=rec[:], in_=den[:])
            rec_bf = recp.tile([1, SL], BF16, name="rec_bf", tag="rec_bf")
            if hh % 2 == 0:
                nc.vector.tensor_scalar_mul(rec_bf[:], rec[:], 1.0)
            else:
                nc.scalar.activation(out=rec_bf[:], in_=rec[:], func=AF.Copy)
            rbps = psS.tile([64, SL], F32, name="rbps", tag="s")
            nc.tensor.matmul(rbps[:], lhsT=ones64[:], rhs=rec_bf[:],
                             start=True, stop=True)
            dst = att_sb[ds(64 * (hh % 2), 64), hc, :]
            if hh % 4 < 2:
                nc.scalar.activation(out=dst, in_=psa[0:HD, :], func=AF.Copy)
            else:
                nc.vector.tensor_scalar_mul(dst, psa[0:HD, :], 1.0)
            nc.vector.tensor_mul(dst, dst, rbps[:])

        if l == 0:
            dump("dbg_k", k_full[:])
            dump("dbg_q", q_sb[:])
            dump("dbg_v", v_full[:])
        AW = 7
        for hh in range(AW):
            scores_head(hh)
        for hh in range(AW, H):
            finish_head(hh - AW)
            scores_head(hh)
        for hh in range(H - AW, H):
            finish_head(hh)
        if l == 0:
            dump("dbg_att", att_sb[:])

        # next-layer projection weights + first W1 eighths (emitted here so
        # their pool-slot waits cannot block attention work on these queues)
        if l + 1 < NL:
            nwk = wqkv.tile([128, NKC, D], BF16, name="wk", tag="wk")
            nc.scalar.dma_start(out=nwk[:], in_=t[f"Wk{l + 1}"][:])
            nwq = wqkv.tile([128, NKC, D], BF16, name="wq", tag="wq")
            nc.scalar.dma_start(out=nwq[:], in_=t[f"Wq{l + 1}"][:])
            nwv = wqkv.tile([128, NKC, D], BF16, name="wv", tag="wv")
            nc.sync.dma_start(out=nwv[:], in_=t[f"Wv{l + 1}"][:])
            nwo = wqkv.tile([128, NKC, D], BF16, name="wo", tag="wk")
            nc.sync.dma_start(out=nwo[:], in_=t[f"Wo{l + 1}"][:])
            wtiles.update(wk=nwk, wv=nwv, wq=nwq, wo=nwo)
        w1q = []

        def w1_load(j):
            w1 = w1p.tile([128, NKC, 512], BF16, name="w1", tag="w1")
            nc.scalar.dma_start(out=w1[:], in_=t[f"W1{l}"][j])
            w1q.append(w1)

        w1_load(0)
        w1_load(1)

        # out-proj + residual
        bo_sb = bias_sb[f"bo{l}"]
        for oc in range(NKC):
            pso = psMM.tile([128, SL], F32, name="pso", tag="mm")
            for kc in range(NKC):
                nc.tensor.matmul(pso[:], lhsT=wo[:, kc, ts(oc, 128)],
                                 rhs=att_sb[:, kc, :],
                                 start=(kc == 0), stop=(kc == NKC - 1))
            nc.vector.scalar_tensor_tensor(
                out=d_bf[:, oc, :], in0=pso[:], scalar=bo_sb[:, oc:oc + 1],
                in1=h_bf[:, oc, :], op0=AX.add, op1=AX.add)

        if l == 0:
            dump("dbg_d1", d_bf[:])
        ln(d_bf, bias_sb[f"ln1s{l}"], bias_sb[f"ln1b{l}"])
        if l == 0:
            dump("dbg_h1", h_bf[:])

        # FFN
        b1_sb = bias_sb[f"b1{l}"]
        for fc in range(NFC):
            if fc % 4 == 0 and fc // 4 + 2 < 8:
                w1_load(fc // 4 + 2)
            w1 = w1q[fc // 4]
            psf = psMM.tile([128, SL], F32, name="psf", tag="mm")
            for kc in range(NKC):
                nc.tensor.matmul(psf[:], lhsT=w1[:, kc, ts(fc % 4, 128)],
                                 rhs=h_bf[:, kc, :],
                                 start=(kc == 0), stop=(kc == NKC - 1))
            nc.scalar.activation(out=g_sb[:, fc, :], in_=psf[:], func=AF.Gelu,
                                 bias=b1_sb[:, fc:fc + 1])
        b2_sb = bias_sb[f"b2{l}"]
        w2t = []
        for oc in range(2):
            w2 = w2p.tile([128, NFC, 128], BF16, name="w2oc", tag="w2oc")
            nc.sync.dma_start(out=w2[:], in_=t[f"W2{l}"][oc])
            w2t.append(w2)
        for oc in range(NKC):
            if oc + 2 < NKC:
                w2 = w2p.tile([128, NFC, 128], BF16, name="w2oc", tag="w2oc")
                nc.sync.dma_start(out=w2[:], in_=t[f"W2{l}"][oc + 2])
                w2t.append(w2)
            psy = psMM.tile([128, SL], F32, name="psy", tag="mm")
            for kc in range(NFC):
                nc.tensor.matmul(psy[:], lhsT=w2t[oc][:, kc, :],
                                 rhs=g_sb[:, kc, :],
                                 start=(kc == 0), stop=(kc == NFC - 1))
            nc.vector.scalar_tensor_tensor(
                out=d_bf[:, oc, :], in0=psy[:], scalar=b2_sb[:, oc:oc + 1],
                in1=h_bf[:, oc, :], op0=AX.add, op1=AX.add)

        if l == 0:
            dump("dbg_g", g_sb[:])
        ln(d_bf, bias_sb[f"ln2s{l}"], bias_sb[f"ln2b{l}"])
        dump("dbg_h2" if l == 0 else "dbg_h3", h_bf[:])

    # ---------------- final: fused LN_f pooling -> per-core partials -------
    mean1, r1, r_b = ln_stats(h_bf)
    mr1 = lnp.tile([1, SL], F32, name="mr1", tag="msq")
    nc.vector.tensor_mul(mr1[:], mean1[:], r1[:])
    csc = constp.tile([1, 1], F32, name="csc", tag="csc")
    nc.vector.reduce_sum(out=csc[:], in_=mr1[:], axis=XL.X)
    for kc in range(NKC):
        t1 = tmpp.tile([128, SL], F32, name="t1", tag="t1")
        nc.vector.scalar_tensor_tensor(
            out=t1[:], in0=h_bf[:, kc, :], scalar=1.0, in1=r_b[:],
            op0=AX.mult, op1=AX.mult, accum_out=hsum[:, kc:kc + 1])
    nc.sync.dma_start(out=out_dram[0:128, :], in_=hsum[:])
    nc.sync.dma_start(out=out_dram[128:129, 0:1], in_=csc[:])
    ctx.close()


# ----------------------------------------------------------------------------
# host side
# ----------------------------------------------------------------------------

def _bf16(x):
    return np.ascontiguousarray(np.asarray(x, dtype=np.float32)).astype(
        ml_dtypes.bfloat16)


def _f32(x):
    return np.ascontiguousarray(np.asarray(x, dtype=np.float32))


def _wtile(a):
    # [D_in, N] -> [128, D_in//128, N] (p, kc, n)
    a = np.asarray(a, np.float32)
    din, n = a.shape
    return _bf16(a.reshape(din // 128, 128, n).transpose(1, 0, 2))


def _btile(a, p=128):
    a = np.asarray(a, np.float32)
    return _f32(a.reshape(-1, p).T)


def _host_weights(inp):
    w = {}
    w["embW"] = _bf16(inp["emb_W"])
    bias_cols = [_btile(inp["emb_b"])]
    for l in range(NL):
        w[f"Wq{l}"] = _wtile(np.asarray(inp["Wq"][l], np.float32) * 0.125)
        w[f"Wk{l}"] = _wtile(inp["Wk"][l])
        w[f"Wv{l}"] = _wtile(inp["Wv"][l])
        w[f"Wo{l}"] = _wtile(inp["Wo"][l])
        w1 = np.asarray(inp["W1"][l], np.float32)
        w[f"W1{l}"] = _bf16(w1.reshape(NKC, 128, 8, 512).transpose(2, 1, 0, 3))
        w2 = np.asarray(inp["W2"][l], np.float32)
        w[f"W2{l}"] = _bf16(w2.reshape(NFC, 128, NKC, 128).transpose(2, 1, 0, 3))
        bias_cols += [
            _btile(np.asarray(inp["bq"][l], np.float32) * 0.125),
            _btile(inp["bk"][l]),
            _btile(inp["bo"][l]),
            _btile(inp["b2"][l]),
            _btile(inp["ln1_s"][l]),
            _btile(inp["ln1_b"][l]),
            _btile(inp["ln2_s"][l]),
            _btile(inp["ln2_b"][l]),
            _btile(inp["b1"][l]),
        ]
    w["biases"] = _f32(np.concatenate(bias_cols, axis=1))
    w["bvpack"] = _bf16(np.concatenate(
        [np.asarray(inp["bv"][l], np.float32) for l in range(NL)])[None, :])
    return w


def kernel(**inputs):
    global _BUILT, LAST_RESULT
    if _BUILT is None:
        _BUILT = _build()
    nc, names = _BUILT

    w = _host_weights(inputs)
    x = np.asarray(inputs["x"], np.float32)  # [4, 3072, 21]
    # host instance norm over time (per batch, channel)
    mu = x.mean(axis=1, keepdims=True)
    xc = x - mu
    sd = np.maximum(np.sqrt(xc.var(axis=1, keepdims=True) + 1e-6), 1e-5)
    xn = xc / sd

    # key-window map in gathered order: [rank0 even windows | rank1 odd]
    wkmap = np.concatenate([np.repeat(np.arange(16) * 2, C),
                            np.repeat(np.arange(16) * 2 + 1, C)])  # [672]
    in_maps = []
    for core in range(8):
        b, parity = core // 2, core % 2
        wins = np.arange(16) * 2 + parity
        xb = xn[b]  # [3072, 21]
        xl = np.empty((P, SL), np.float32)
        for i, wn in enumerate(wins):
            xl[:, i * C:(i + 1) * C] = xb[wn * P:(wn + 1) * P, :]
        wqw = np.repeat(wins, C)
        mask = (wkmap[:, None] <= wqw[None, :]).astype(np.float32)  # [672,336]
        packed = np.zeros((112, EW), np.float32)
        for cc in range(6):
            tqs, suf = SUF0[cc], SUF[cc]
            packed[:, EOFF[cc]:EOFF[cc] + suf] = \
                mask[cc * 112:(cc + 1) * 112, tqs:tqs + suf]
        m = dict(w)
        m["xloc"] = _bf16(xl)
        m["maskP"] = _bf16(packed)
        in_maps.append(m)

    res = run_bass_kernel_spmd(nc, in_maps, core_ids=list(range(8)))
    LAST_RESULT = res

    # host finish: combine pair partials, fold lnf + head + MLP (fp32)
    lnf_s = np.asarray(inputs["lnf_s"], np.float32)
    lnf_b = np.asarray(inputs["lnf_b"], np.float32)
    hW = np.asarray(inputs["head_W"], np.float32)
    hb = np.asarray(inputs["head_b"], np.float32)
    c1W, c1b = (np.asarray(inputs[k], np.float32) for k in ("c1_W", "c1_b"))
    c2W, c2b = (np.asarray(inputs[k], np.float32) for k in ("c2_W", "c2_b"))
    c3W, c3b = (np.asarray(inputs[k], np.float32) for k in ("c3_W", "c3_b"))
    logits = np.empty((B, 2), np.float32)
    for b in range(B):
        o0 = np.asarray(res.results[2 * b]["out"], np.float32)
        o1 = np.asarray(res.results[2 * b + 1]["out"], np.float32)
        hs = (o0 + o1)
        dvec = hs[0:128, :].T.reshape(-1)        # d = kc*128 + p
        cs = hs[128, 0]
        pooled = lnf_s * (dvec - cs) + S * lnf_b
        feat = (pooled / S) @ hW + hb
        feat = np.nan_to_num(feat, nan=0.0, posinf=0.0, neginf=0.0)
        z1 = np.maximum(feat @ c1W + c1b, 0.0)
        z2 = np.maximum(z1 @ c2W + c2b, 0.0)
        logits[b] = z2 @ c3W + c3b
    return logits
